# revision 1
# baseline (speedup 1.0000x reference)
"""GAT 2-layer network on 8 Trainium2 NeuronCores.

Strategy (edge-parallel, per the sharding hint "partition edges, replicate
node features"):
  - Nodes are sorted by in-degree and packed into 128-node tiles; tiles are
    dealt round-robin onto the 8 cores so every core runs the identical
    instruction stream (SPMD) over a shared per-step K schedule.
  - All FLOPs run on device across 3 launches:
      K1: xp1 = x @ W1 plus per-head attention dot products (s1, ad1).
      K2: per dst-tile segment softmax + message aggregation for layer 1,
          ELU, then xp2 = h @ W2ext (fused) -> layer-2 node table.
      K3: layer-2 segment softmax + aggregation + bias + log_softmax.
  - Between launches the host only does index-based data movement: it
    replicates the device-computed per-node tables into per-edge-slot
    streams (degree-padded, p-major) so each device step reads purely
    sequential DMA. No floating-point math happens on the host.
"""

import os
import sys

for _p in ("/opt/trn_rl_repo", "/root/.axon_site/_ro/trn_rl_repo"):
    if os.path.isdir(_p) and _p not in sys.path:
        sys.path.insert(0, _p)

import numpy as np

import concourse.bacc as bacc
import concourse.bass as bass
import concourse.tile as tile
from concourse import mybir
from concourse.bass_utils import run_bass_kernel_spmd

F32 = mybir.dt.float32
AF = mybir.ActivationFunctionType
ALU = mybir.AluOpType
AX = mybir.AxisListType

N = 100000
E = 1600000
F_IN = 256
H1, D1 = 8, 8
HD1 = H1 * D1          # 64
D2 = 16                # H2 = 1
NEG = 0.2
NC = 8
P = 128
TILES = 784            # ceil(100000 / 128) rounded up to a multiple of 8
STEPS = TILES // NC    # 98
NPC = STEPS * P        # 12544 node rows handled per core in K1
PADS = -1.0e38         # sentinel: exp(lrelu(PADS + ad)) == 0 exactly

TRACE = False          # test.py flips this for NTFF profiling
SIM = False            # run through CoreSim instead of hardware
SIM_CORES = None       # e.g. [0] to only simulate core 0
LAST_EXEC_NS = []      # per-launch exec_time_ns when TRACE


def _run(nc, in_maps, tag):
    if SIM:
        from concourse.bass_interp import CoreSim

        outs = []
        cores = range(NC) if SIM_CORES is None else SIM_CORES
        for c in range(NC):
            if c not in cores:
                outs.append(outs[-1] if outs else {})
                continue
            sim = CoreSim(nc, trace=False)
            for k, v in in_maps[c].items():
                sim.tensor(k)[:] = v
            sim.simulate(check_with_hw=False)
            onames = [
                a.memorylocations[0].name
                for a in nc.m.functions[0].allocations
                if isinstance(a, mybir.MemoryLocationSet) and a.kind == "ExternalOutput"
            ]
            outs.append({k: np.array(sim.tensor(k)) for k in onames})
        return outs
    if TRACE:
        import hookfix  # noqa: F401  (registers antenv.axon_hooks)

        hookfix.install()
    res = run_bass_kernel_spmd(nc, in_maps, list(range(NC)), trace=TRACE)
    if TRACE:
        LAST_EXEC_NS.append((tag, res.exec_time_ns))
    return res.results


def _bc(ap, shape):
    """Broadcast the free dims of `ap` to `shape` (partition dim must already
    match).  Target dims are matched against source free dims right-to-left;
    size-1 source dims and unmatched target dims become step-0 (broadcast)."""
    src = ap.ap
    assert src[0][1] == shape[0], (src, shape)
    sdims = list(src[1:])
    res = []
    si = len(sdims) - 1
    for ti in range(len(shape) - 1, 0, -1):
        if si >= 0 and sdims[si][1] == shape[ti]:
            res.append(sdims[si])
            si -= 1
        elif si >= 0 and sdims[si][1] == 1:
            res.append([0, shape[ti]])
            si -= 1
        else:
            res.append([0, shape[ti]])
    assert si < 0, (src, shape)
    return bass.AP(tensor=ap.tensor, offset=ap.offset, ap=[src[0]] + res[::-1])


def _tail0(ap, n):
    """Append a trailing step-0 (broadcast) dim of size n."""
    return bass.AP(tensor=ap.tensor, offset=ap.offset, ap=list(ap.ap) + [[0, n]])


def _mid0(ap, pos, n):
    """Insert a step-0 (broadcast) dim of size n at free-dim position pos
    (ap.ap index pos, counting the partition dim as 0)."""
    dims = list(ap.ap)
    return bass.AP(
        tensor=ap.tensor, offset=ap.offset, ap=dims[:pos] + [[0, n]] + dims[pos:]
    )


def _rep_row(nc, pool, dram_t, nparts, cols, tag):
    """DMA-replicate a flat `cols`-element DRAM tensor across `nparts`
    partitions (engines cannot broadcast across partitions themselves)."""
    tl = pool.tile([nparts, cols], F32, tag=tag)
    src = bass.AP(tensor=dram_t[:].tensor, offset=0, ap=[[0, nparts], [1, cols]])
    nc.sync.dma_start(tl[:], src)
    return tl


# --------------------------------------------------------------------------
# K1: node tables.  out column-major xq1T [80, NPC] per core:
#     rows 0:64 xp1 = x @ W1, 64:72 s1 (att_src dot), 72:80 ad1 (att_dst dot)
# --------------------------------------------------------------------------
def build_k1():
    nc = bacc.Bacc("TRN2", target_bir_lowering=False, debug=False, num_devices=NC)
    xT = nc.dram_tensor("xT", [F_IN, NPC], F32, kind="ExternalInput")
    w1 = nc.dram_tensor("w1", [F_IN, HD1], F32, kind="ExternalInput")
    as1 = nc.dram_tensor("as1", [H1, D1], F32, kind="ExternalInput")
    ad1 = nc.dram_tensor("ad1", [H1, D1], F32, kind="ExternalInput")
    out = nc.dram_tensor("xq1T", [80, NPC], F32, kind="ExternalOutput")

    with tile.TileContext(nc) as tc:
        with (
            tc.tile_pool(name="pro", bufs=1) as pro,
            tc.tile_pool(name="io", bufs=3) as io,
            tc.tile_pool(name="ps", bufs=4, space="PSUM") as ps,
        ):
            w1sb = pro.tile([P, 2, HD1], F32)
            nc.sync.dma_start(w1sb[:], w1[:].rearrange("(c p) d -> p c d", p=P))
            asr = _rep_row(nc, pro, as1, P, HD1, "asr")
            adr = _rep_row(nc, pro, ad1, P, HD1, "adr")

            # w_s1[f, h] = sum_d W1[f, h*8+d] * att_src1[h, d]; same for dst
            wext = pro.tile([P, 2, 80], F32)
            nc.scalar.copy(wext[:, :, 0:HD1], w1sb[:])
            for att, lo in ((asr, 64), (adr, 72)):
                tmp = pro.tile([P, 2, HD1], F32, tag="k1tmp")
                nc.vector.tensor_tensor(
                    tmp[:], w1sb[:], _bc(att[:], [P, 2, HD1]), op=ALU.mult
                )
                nc.vector.tensor_reduce(
                    wext[:, :, lo : lo + 8],
                    tmp[:].rearrange("p c (h d) -> p c h d", d=D1),
                    axis=AX.X,
                    op=ALU.add,
                )

            xTr = xT[:].rearrange("(c p) n -> p c n", p=P)
            GT = 4                                  # node-tiles per matmul
            for t0 in range(0, STEPS, GT):
                g = min(GT, STEPS - t0)
                W = g * P
                xt = io.tile([P, 2, GT * P], F32, tag="xt")
                nc.sync.dma_start(xt[:, :, 0:W], xTr[:, :, t0 * P : t0 * P + W])
                pt = ps.tile([80, GT * P], F32, tag="k1ps")
                nc.tensor.matmul(
                    pt[:, 0:W], lhsT=wext[:, 0, :], rhs=xt[:, 0, 0:W],
                    start=True, stop=False,
                )
                nc.tensor.matmul(
                    pt[:, 0:W], lhsT=wext[:, 1, :], rhs=xt[:, 1, 0:W],
                    start=False, stop=True,
                )
                ot = io.tile([80, GT * P], F32, tag="k1o")
                nc.vector.tensor_copy(ot[:, 0:W], pt[:, 0:W])
                nc.sync.dma_start(out[:, t0 * P : t0 * P + W], ot[:, 0:W])
    nc.compile()
    return nc


# --------------------------------------------------------------------------
# K2: layer-1 edge aggregation + ELU + fused xp2/s2/ad2 table.
#   EV1 row (72 f32): [xp1(64) | s1(8)] for the slot's src node (PADS rows
#   have s1 = -1e38 so exp()==0).  p-major slots: slot = base + p*K + k.
#   out t2T [18, NPC] column-major: rows 0:16 xp2, 16 s2, 17 ad2.
# --------------------------------------------------------------------------
def build_k2(groups, k_tile):
    slots = P * sum(g * kb for _, g, kb in groups)
    nc = bacc.Bacc("TRN2", target_bir_lowering=False, debug=False, num_devices=NC)
    evs = nc.dram_tensor("ev1s", [8 * slots], F32, kind="ExternalInput")
    evx = nc.dram_tensor("ev1x", [64 * slots], F32, kind="ExternalInput")
    adt = nc.dram_tensor("adR", [NPC, H1], F32, kind="ExternalInput")
    w2 = nc.dram_tensor("w2", [HD1, D2], F32, kind="ExternalInput")
    as2 = nc.dram_tensor("as2", [1, D2], F32, kind="ExternalInput")
    ad2 = nc.dram_tensor("ad2", [1, D2], F32, kind="ExternalInput")
    b1t = nc.dram_tensor("b1", [HD1], F32, kind="ExternalInput")
    out = nc.dram_tensor("t2T", [18, NPC], F32, kind="ExternalOutput")

    from concourse.masks import make_identity

    with tile.TileContext(nc) as tc:
        with (
            tc.tile_pool(name="pro", bufs=1) as pro,
            tc.tile_pool(name="io", bufs=2) as io,
            tc.tile_pool(name="wk", bufs=2) as wk,
            tc.tile_pool(name="ps", bufs=2, space="PSUM") as ps,
        ):
            w2sb = pro.tile([HD1, D2], F32)
            nc.sync.dma_start(w2sb[:], w2[:])
            a2s = _rep_row(nc, pro, as2, HD1, D2, "a2s")
            a2d = _rep_row(nc, pro, ad2, HD1, D2, "a2d")
            b1r = _rep_row(nc, pro, b1t, P, HD1, "b1r")
            ident = pro.tile([P, P], F32)
            make_identity(nc, ident[:])
            c_eps = pro.tile([P, 1], F32)
            nc.vector.memset(c_eps[:], 1e-16)
            c_m1 = pro.tile([P, 1], F32)
            nc.vector.memset(c_m1[:], -1.0)

            # W2ext [64, 18] = [W2 | W2@att_src2 | W2@att_dst2]
            w2e = pro.tile([HD1, 18], F32)
            nc.scalar.copy(w2e[:, 0:D2], w2sb[:])
            for att, col in ((a2s, 16), (a2d, 17)):
                tmp2 = pro.tile([HD1, D2], F32, tag="k2tmp")
                nc.vector.tensor_tensor(tmp2[:], w2sb[:], att[:], op=ALU.mult)
                nc.vector.tensor_reduce(
                    w2e[:, col : col + 1], tmp2[:], axis=AX.X, op=ALU.add
                )

            base = 0
            for t0, G, K in groups:
                est = io.tile([P, G, 8 * K], F32, tag="evs")
                nc.sync.dma_start(
                    est[:],
                    evs[8 * base : 8 * (base + P * G * K)].rearrange(
                        "(p g f) -> p g f", g=G, f=8 * K
                    ),
                )
                ext = io.tile([P, G, 64 * K], F32, tag="evx")
                nc.sync.dma_start(
                    ext[:],
                    evx[64 * base : 64 * (base + P * G * K)].rearrange(
                        "(p g f) -> p g f", g=G, f=64 * K
                    ),
                )
                base += P * G * K
                adv = io.tile([P, G, H1], F32, tag="ad")
                nc.sync.dma_start(
                    adv[:],
                    adt[t0 * P : (t0 + G) * P, :].rearrange("(g p) h -> p g h", p=P),
                )

                # ex = exp(lrelu(s1+ad1)) = exp(0.2 e) * exp(relu(0.8 e))
                e = wk.tile([P, G, H1, K], F32, tag="e")
                nc.vector.tensor_tensor(
                    e[:],
                    est[:].rearrange("p g (h k) -> p g h k", k=K),
                    _tail0(adv[:], K),
                    op=ALU.add,
                )
                ea = wk.tile([P, G, H1, K], F32, tag="ea")
                nc.scalar.activation(ea[:], e[:], AF.Exp, scale=NEG)
                eb = wk.tile([P, G, H1, K], F32, tag="eb")
                nc.scalar.activation(eb[:], e[:], AF.Relu, scale=1.0 - NEG)
                nc.scalar.activation(eb[:], eb[:], AF.Exp)
                ex = ea
                nc.vector.tensor_tensor(ex[:], ea[:], eb[:], op=ALU.mult)

                # denom + reciprocal
                dn = wk.tile([P, G, H1], F32, tag="dn")
                nc.vector.tensor_reduce(dn[:], ex[:], axis=AX.X, op=ALU.add)
                inv = wk.tile([P, G, H1], F32, tag="inv")
                nc.scalar.activation(inv[:], dn[:], AF.Identity, bias=c_eps[:])
                nc.vector.reciprocal(inv[:], inv[:])

                # msg[p,(g h),d,k] = ex * xp — one TT per group, with the k
                # axis split between gpsimd (bulk) and DVE (remainder).
                msg = wk.tile([P, G * H1, D1, K], F32, tag="msg")
                xpall = ext[:].rearrange("p g (h d k) -> p (g h) d k", d=D1, k=K)
                exall = ex[:].rearrange("p g h k -> p (g h) k")
                nc.vector.tensor_tensor(
                    msg[:], xpall[:], _mid0(exall[:], 2, D1), op=ALU.mult
                )
                agg = wk.tile([P, G, H1, D1], F32, tag="agg")
                nc.vector.tensor_reduce(
                    agg[:].rearrange("p g h d -> p (g h) d"),
                    msg[:],
                    axis=AX.X,
                    op=ALU.add,
                )

                # h = elu(agg * inv + b1)
                hsb = wk.tile([P, G, HD1], F32, tag="hsb")
                nc.vector.tensor_tensor(
                    hsb[:].rearrange("p g (h d) -> p g h d", d=D1),
                    agg[:],
                    _tail0(inv[:], D1),
                    op=ALU.mult,
                )
                nc.vector.tensor_tensor(
                    hsb[:], hsb[:], _bc(b1r[:], [P, G, HD1]), op=ALU.add
                )
                hpos = wk.tile([P, G, HD1], F32, tag="hpos")
                nc.scalar.activation(hpos[:], hsb[:], AF.Relu)
                nc.vector.tensor_tensor(hsb[:], hsb[:], hpos[:], op=ALU.subtract)
                nc.scalar.activation(hsb[:], hsb[:], AF.Exp)  # exp(min(h,0))
                nc.vector.tensor_tensor(hsb[:], hsb[:], hpos[:], op=ALU.add)
                nc.scalar.activation(hsb[:], hsb[:], AF.Identity, bias=c_m1[:])

                # xp2/s2/ad2 via per-tile transpose + matmul
                shT = wk.tile([HD1, G, P], F32, tag="shT")
                pt2 = ps.tile([18, G, P], F32, tag="pt2")
                for g in range(G):
                    phT = ps.tile([HD1, P], F32, tag="phT")
                    nc.tensor.transpose(phT[:], hsb[:, g, :], ident[:])
                    nc.scalar.copy(shT[:, g, :], phT[:])
                    nc.tensor.matmul(
                        pt2[:, g, :], lhsT=w2e[:], rhs=shT[:, g, :],
                        start=True, stop=True,
                    )
                st2 = io.tile([18, G, P], F32, tag="st2")
                nc.scalar.copy(st2[:], pt2[:])
                nc.sync.dma_start(
                    out[:, t0 * P : (t0 + G) * P],
                    st2[:].rearrange("r g n -> r (g n)"),
                )
    nc.compile()
    return nc


# --------------------------------------------------------------------------
# K3: layer-2 edge aggregation + bias + log_softmax.
#   EV2 row (18 f32): [xp2(16) | s2(1) | pad] for the slot's src node.
# --------------------------------------------------------------------------
def build_k3(groups):
    tot = 17 * P * sum(g * kb for _, g, kb in groups)
    nc = bacc.Bacc("TRN2", target_bir_lowering=False, debug=False, num_devices=NC)
    ev = nc.dram_tensor("ev2", [tot], F32, kind="ExternalInput")
    adt = nc.dram_tensor("ad2R", [NPC, 1], F32, kind="ExternalInput")
    b2t = nc.dram_tensor("b2", [D2], F32, kind="ExternalInput")
    out = nc.dram_tensor("o3", [NPC, D2], F32, kind="ExternalOutput")

    with tile.TileContext(nc) as tc:
        with (
            tc.tile_pool(name="pro", bufs=1) as pro,
            tc.tile_pool(name="io", bufs=3) as io,
            tc.tile_pool(name="wk", bufs=2) as wk,
        ):
            b2r = _rep_row(nc, pro, b2t, P, D2, "b2r")

            base = 0
            for t0, G, K in groups:
                evt = io.tile([P, G, 17 * K], F32, tag="ev")
                nc.sync.dma_start(
                    evt[:],
                    ev[base : base + P * G * 17 * K].rearrange(
                        "(p g f) -> p g f", g=G, f=17 * K
                    ),
                )
                base += P * G * 17 * K
                adv = io.tile([P, G, 1], F32, tag="ad")
                nc.sync.dma_start(
                    adv[:],
                    adt[t0 * P : (t0 + G) * P, :].rearrange("(g p) o -> p g o", p=P),
                )

                e = wk.tile([P, G, K], F32, tag="e")
                nc.vector.tensor_tensor(
                    e[:], evt[:, :, 16 * K : 17 * K], _bc(adv[:], [P, G, K]), op=ALU.add
                )
                et = wk.tile([P, G, K], F32, tag="et")
                nc.vector.tensor_scalar_mul(et[:], e[:], NEG)
                nc.vector.tensor_tensor(e[:], e[:], et[:], op=ALU.max)
                nc.scalar.activation(e[:], e[:], AF.Exp)

                dn = wk.tile([P, G], F32, tag="dn")
                nc.vector.tensor_reduce(dn[:], e[:], axis=AX.X, op=ALU.add)
                nc.vector.tensor_scalar_add(dn[:], dn[:], 1e-16)
                inv = wk.tile([P, G], F32, tag="inv")
                nc.vector.reciprocal(inv[:], dn[:])

                msg = wk.tile([P, G, D2, K], F32, tag="msg")
                nc.vector.tensor_tensor(
                    msg[:],
                    evt[:, :, 0 : 16 * K].rearrange("p g (d k) -> p g d k", k=K),
                    _mid0(e[:], 2, D2),
                    op=ALU.mult,
                )
                o = wk.tile([P, G, D2], F32, tag="o")
                nc.vector.tensor_reduce(o[:], msg[:], axis=AX.X, op=ALU.add)
                nc.vector.tensor_tensor(o[:], o[:], _tail0(inv[:], D2), op=ALU.mult)
                nc.vector.tensor_tensor(
                    o[:], o[:], _bc(b2r[:], [P, G, D2]), op=ALU.add
                )

                # log_softmax over the 16 classes
                nm = wk.tile([P, G], F32, tag="nm")
                nc.vector.tensor_reduce(nm[:], o[:], axis=AX.X, op=ALU.max, negate=True)
                nc.vector.tensor_tensor(o[:], o[:], _tail0(nm[:], D2), op=ALU.add)
                exq = wk.tile([P, G, D2], F32, tag="exq")
                nc.scalar.activation(exq[:], o[:], AF.Exp)
                ss = wk.tile([P, G], F32, tag="ss")
                nc.vector.tensor_reduce(ss[:], exq[:], axis=AX.X, op=ALU.add)
                nc.scalar.activation(ss[:], ss[:], AF.Ln)
                nc.vector.tensor_tensor(o[:], o[:], _tail0(ss[:], D2), op=ALU.subtract)

                nc.sync.dma_start(
                    out[t0 * P : (t0 + G) * P, :].rearrange("(g p) f -> p g f", p=P),
                    o[:],
                )
    nc.compile()
    return nc


# --------------------------------------------------------------------------
# Host orchestration
# --------------------------------------------------------------------------
def _make_groups(k_step, gmax, slot_budget):
    """Greedy: grow the group while tiles*K stays under slot_budget."""
    groups = []
    t0 = 0
    while t0 < STEPS:
        g = 1
        kb = max(int(k_step[t0]), 1)
        while (
            t0 + g < STEPS
            and g < gmax
            and (g + 1) * max(kb, int(k_step[t0 + g])) <= slot_budget
        ):
            kb = max(kb, int(k_step[t0 + g]))
            g += 1
        groups.append((t0, g, kb))
        t0 += g
    return groups


def _build_slots(groups, spos_node, deg, estart, src_by_dst):
    """slot -> src node id (N = pad) per core; layout per group is p-major:
    slot = base + p*(G*K) + g*K + k."""
    tot = sum(P * g * kb for _, g, kb in groups)
    slot = np.full((NC, tot), N, dtype=np.int64)
    arangeP = np.arange(P)
    for c in range(NC):
        base = 0
        for t0, g, kb in groups:
            for gi in range(g):
                T = (t0 + gi) * NC + c
                nodes = spos_node[T * P : (T + 1) * P]
                valid = nodes >= 0
                nv = nodes[valid]
                if nv.size == 0:
                    continue
                d = deg[nv]
                rowstart = base + arangeP[valid] * (g * kb) + gi * kb
                totd = int(d.sum())
                if totd == 0:
                    continue
                rep_row = np.repeat(rowstart, d)
                rep_cum = np.repeat(np.cumsum(d) - d, d)
                intra = np.arange(totd) - rep_cum
                rep_est = np.repeat(estart[nv], d)
                slot[c, rep_row + intra] = src_by_dst[rep_est + intra]
            base += P * g * kb
    return slot


def kernel(x, edge_index, W1, att_src1, att_dst1, b1, W2, att_src2, att_dst2, b2):
    x = np.asarray(x, dtype=np.float32)
    edge_index = np.asarray(edge_index)
    W1 = np.asarray(W1, dtype=np.float32)
    att_src1 = np.asarray(att_src1, dtype=np.float32)
    att_dst1 = np.asarray(att_dst1, dtype=np.float32)
    b1 = np.asarray(b1, dtype=np.float32)
    W2 = np.asarray(W2, dtype=np.float32)
    att_src2 = np.asarray(att_src2, dtype=np.float32).reshape(1, D2)
    att_dst2 = np.asarray(att_dst2, dtype=np.float32).reshape(1, D2)
    b2 = np.asarray(b2, dtype=np.float32)

    src = edge_index[0].astype(np.int64)
    dst = edge_index[1].astype(np.int64)

    # ---- schedule: degree-sorted tiles, round-robin dealt across cores ----
    deg = np.bincount(dst, minlength=N)
    order = np.argsort(deg, kind="stable")          # sorted-node space -> node id
    eo = np.argsort(dst, kind="stable")             # edges sorted by dst
    src_by_dst = src[eo]
    estart = np.zeros(N + 1, dtype=np.int64)
    estart[1:] = np.cumsum(deg)

    spos_node = np.full(TILES * P, -1, dtype=np.int64)
    spos_node[:N] = order
    sdeg = np.zeros(TILES * P, dtype=np.int64)
    sdeg[:N] = deg[order]
    tile_max = sdeg.reshape(TILES, P).max(axis=1)
    k_step = np.maximum(tile_max.reshape(STEPS, NC).max(axis=1), 1)  # [STEPS]

    groups2 = _make_groups(k_step, 4, 96)
    groups3 = _make_groups(k_step, 8, 200)
    slots2 = _build_slots(groups2, spos_node, deg, estart, src_by_dst)
    slots3 = _build_slots(groups3, spos_node, deg, estart, src_by_dst)
    ad_rows = np.where(spos_node < 0, N, spos_node)  # [TILES*P] node per row
    # per-core view: row t*128+p of core c <-> sorted pos (t*NC+c)*128+p
    ad_rows = (
        ad_rows.reshape(STEPS, NC, P).transpose(1, 0, 2).reshape(NC, NPC)
    )

    # ---- K1: node tables ----
    xpad = np.zeros((NC * NPC, F_IN), dtype=np.float32)
    xpad[:N] = x
    nc1 = build_k1()
    in1 = [
        {
            "xT": np.ascontiguousarray(xpad[c * NPC : (c + 1) * NPC].T),
            "w1": W1,
            "as1": att_src1,
            "ad1": att_dst1,
        }
        for c in range(NC)
    ]
    r1 = _run(nc1, in1, "k1")
    xq1 = np.empty((NC * NPC + 1, 80), dtype=np.float32)
    for c in range(NC):
        xq1[c * NPC : (c + 1) * NPC] = r1[c]["xq1T"].T
    xq1[-1] = 0.0
    xq1[-1, 64:72] = PADS                           # pad row: s1 = -1e38

    # ---- K2: layer 1 ----
    nc2 = build_k2(groups2, k_step)
    pad2 = np.where(slots2 >= N, NC * NPC, slots2)

    def _soa1(c):
        """Two streams, per (group, p, g) blocks, k innermost:
        s1 (8,K) and xp1 (8,8,K)."""
        rows = xq1[pad2[c], 0:72]
        outs = np.empty(rows.shape[0] * 8, dtype=np.float32)
        outx = np.empty(rows.shape[0] * 64, dtype=np.float32)
        bs = 0
        for _t0, g, kb in groups2:
            n = P * g * kb
            arr = rows[bs : bs + n].reshape(P, g, kb, 72)
            outs[bs * 8 : (bs + n) * 8] = (
                arr[..., 64:72].transpose(0, 1, 3, 2).ravel()
            )
            outx[bs * 64 : (bs + n) * 64] = (
                arr[..., 0:64].reshape(P, g, kb, 8, 8).transpose(0, 1, 3, 4, 2).ravel()
            )
            bs += n
        return outs, outx

    soa1 = [_soa1(c) for c in range(NC)]
    in2 = [
        {
            "ev1s": soa1[c][0],
            "ev1x": soa1[c][1],
            "adR": xq1[np.where(ad_rows[c] >= N, NC * NPC, ad_rows[c]), 72:80],
            "w2": W2,
            "as2": att_src2,
            "ad2": att_dst2,
            "b1": b1,
        }
        for c in range(NC)
    ]
    r2 = _run(nc2, in2, "k2")

    # reassemble layer-2 node table in original-node space
    t2 = np.zeros((N + 1, 18), dtype=np.float32)
    t2[N, 16] = PADS                                # pad row: s2 = -1e38
    for c in range(NC):
        cols = r2[c]["t2T"]                         # [18, NPC]
        rows = cols.T.reshape(STEPS, P, 18)
        for t in range(STEPS):
            T = t * NC + c
            nodes = spos_node[T * P : (T + 1) * P]
            valid = nodes >= 0
            t2[nodes[valid]] = rows[t][valid]

    # ---- K3: layer 2 ----
    nc3 = build_k3(groups3)
    pad3 = np.where(slots3 >= N, N, slots3)

    def _soa2(c):
        """Per (group, p, g) blocks: [xp2 (16,K) | s2 (K)], k innermost."""
        rows = t2[pad3[c]]
        out = np.empty(rows.shape[0] * 17, dtype=np.float32)
        bs = 0
        bf = 0
        for _t0, g, kb in groups3:
            n = P * g * kb
            arr = rows[bs : bs + n].reshape(P, g, kb, 18)
            xp = arr[..., 0:16].transpose(0, 1, 3, 2).reshape(P, g, 16 * kb)
            s = arr[..., 16].reshape(P, g, kb)
            out[bf : bf + n * 17] = np.concatenate([xp, s], axis=2).ravel()
            bs += n
            bf += n * 17
        return out

    in3 = [
        {
            "ev2": _soa2(c),
            "ad2R": t2[np.where(ad_rows[c] >= N, N, ad_rows[c]), 17:18],
            "b2": b2,
        }
        for c in range(NC)
    ]
    r3 = _run(nc3, in3, "k3")

    outp = np.zeros((N, D2), dtype=np.float32)
    for c in range(NC):
        o = r3[c]["o3"].reshape(STEPS, P, D2)
        for t in range(STEPS):
            T = t * NC + c
            nodes = spos_node[T * P : (T + 1) * P]
            valid = nodes >= 0
            outp[nodes[valid]] = o[t][valid]
    return outp



# revision 3
# speedup vs baseline: 1.4128x; 1.4128x over previous
"""GAT 2-layer network on 8 Trainium2 NeuronCores.

Strategy (edge-parallel, per the sharding hint "partition edges, replicate
node features"):
  - Nodes are sorted by in-degree and packed into 128-node tiles; tiles are
    dealt round-robin onto the 8 cores so every core runs the identical
    instruction stream (SPMD) over a shared per-step K schedule.
  - All FLOPs run on device across 3 launches:
      K1: xp1 = x @ W1 plus per-head attention dot products (s1, ad1),
          emitted in bf16.
      K2: per dst-tile segment softmax + message aggregation for layer 1,
          ELU, then xp2 = h @ W2ext (fused) -> layer-2 node table (bf16).
      K3: layer-2 segment softmax + aggregation + bias + log_softmax (fp32
          out).
  - Between launches the host only does index-based data movement: it
    replicates the device-computed per-node tables into per-edge-slot
    streams (degree-padded, p-major, bf16 moved as uint16) so each device
    step reads one contiguous DMA per group.  No floating-point math
    happens on the host.
  - Perf notes vs the first version of this kernel: all edge streams are
    bf16 (halves HBM traffic AND unlocks the DVE 2x_1p mode for
    tensor_tensor), segment reductions use in-place halving trees of
    2x-mode TT adds instead of the never-accelerated tensor_reduce, the
    leaky-relu+exp runs on the Scalar engine (Prelu+Exp share one
    activation table set), and each group's inputs arrive as a single
    contiguous DMA.
"""

import os
import sys

for _p in ("/opt/trn_rl_repo", "/root/.axon_site/_ro/trn_rl_repo"):
    if os.path.isdir(_p) and _p not in sys.path:
        sys.path.insert(0, _p)

import ml_dtypes
import numpy as np

import concourse.bacc as bacc
import concourse.bass as bass
import concourse.tile as tile
from concourse import mybir
from concourse.bass_utils import run_bass_kernel_spmd

F32 = mybir.dt.float32
BF16 = mybir.dt.bfloat16
AF = mybir.ActivationFunctionType
ALU = mybir.AluOpType
AX = mybir.AxisListType

N = 100000
E = 1600000
F_IN = 256
H1, D1 = 8, 8
HD1 = H1 * D1          # 64
D2 = 16                # H2 = 1
NEG = 0.2
NC = 8
P = 128
TILES = 784            # ceil(100000 / 128) rounded up to a multiple of 8
STEPS = TILES // NC    # 98
NPC = STEPS * P        # 12544 node rows handled per core in K1
PADS = -1.0e38         # sentinel: exp(lrelu(PADS + ad)) == 0 exactly

USE_PRELU = True       # Prelu not implemented by CoreSim; HW supports it
TRACE = False          # test.py flips this for NTFF profiling
SIM = False            # run through CoreSim instead of hardware
SIM_CORES = None       # e.g. [0] to only simulate core 0
LAST_EXEC_NS = []      # per-launch exec_time_ns when TRACE

BF_U16 = np.dtype(ml_dtypes.bfloat16)


def _bf(x):
    """View a uint16 array as bfloat16 (no conversion)."""
    return np.asarray(x).view(BF_U16)


def _u16(x):
    """View a bfloat16-ish 2-byte array as uint16 (no conversion)."""
    x = np.asarray(x)
    assert x.dtype.itemsize == 2, x.dtype
    return x.view(np.uint16)


def _run(nc, in_maps, tag):
    if SIM:
        from concourse.bass_interp import CoreSim

        outs = []
        cores = range(NC) if SIM_CORES is None else SIM_CORES
        for c in range(NC):
            if c not in cores:
                outs.append(outs[-1] if outs else {})
                continue
            sim = CoreSim(nc, trace=False)
            for k, v in in_maps[c].items():
                sim.tensor(k)[:] = v
            sim.simulate(check_with_hw=False)
            onames = [
                a.memorylocations[0].name
                for a in nc.m.functions[0].allocations
                if isinstance(a, mybir.MemoryLocationSet) and a.kind == "ExternalOutput"
            ]
            outs.append({k: np.array(sim.tensor(k)) for k in onames})
        return outs
    if TRACE:
        import hookfix  # noqa: F401  (registers antenv.axon_hooks)

        hookfix.install()
    res = run_bass_kernel_spmd(nc, in_maps, list(range(NC)), trace=TRACE)
    if TRACE:
        LAST_EXEC_NS.append((tag, res.exec_time_ns))
    return res.results


def _bc(ap, shape):
    """Broadcast the free dims of `ap` to `shape` (partition dim must already
    match).  Target dims are matched against source free dims right-to-left;
    size-1 source dims and unmatched target dims become step-0 (broadcast)."""
    src = ap.ap
    assert src[0][1] == shape[0], (src, shape)
    sdims = list(src[1:])
    res = []
    si = len(sdims) - 1
    for ti in range(len(shape) - 1, 0, -1):
        if si >= 0 and sdims[si][1] == shape[ti]:
            res.append(sdims[si])
            si -= 1
        elif si >= 0 and sdims[si][1] == 1:
            res.append([0, shape[ti]])
            si -= 1
        else:
            res.append([0, shape[ti]])
    assert si < 0, (src, shape)
    return bass.AP(tensor=ap.tensor, offset=ap.offset, ap=[src[0]] + res[::-1])


def _tail0(ap, n):
    """Append a trailing step-0 (broadcast) dim of size n."""
    return bass.AP(tensor=ap.tensor, offset=ap.offset, ap=list(ap.ap) + [[0, n]])


def _mid0(ap, pos, n):
    """Insert a step-0 (broadcast) dim of size n at free-dim position pos
    (ap.ap index pos, counting the partition dim as 0)."""
    dims = list(ap.ap)
    return bass.AP(
        tensor=ap.tensor, offset=ap.offset, ap=dims[:pos] + [[0, n]] + dims[pos:]
    )


def _rep_row(nc, pool, dram_t, nparts, cols, tag):
    """DMA-replicate a flat `cols`-element DRAM tensor across `nparts`
    partitions (engines cannot broadcast across partitions themselves)."""
    tl = pool.tile([nparts, cols], F32, tag=tag)
    src = bass.AP(tensor=dram_t[:].tensor, offset=0, ap=[[0, nparts], [1, cols]])
    nc.sync.dma_start(tl[:], src)
    return tl


def _tree_reduce(nc, msg_ap_fn, out_ap, kb):
    """Reduce the innermost (k) axis of a degree-padded block with in-place
    halving TT adds (2x-mode in bf16) while the width stays a multiple of 4,
    then one tensor_reduce over the tail.  `msg_ap_fn(lo, w)` returns the AP
    covering k in [lo, lo+w)."""
    w = kb
    while w > 2 and w % 4 == 0:
        h2 = w // 2
        nc.vector.tensor_tensor(
            msg_ap_fn(0, h2), msg_ap_fn(0, h2), msg_ap_fn(h2, h2), op=ALU.add
        )
        w = h2
    nc.vector.tensor_reduce(out_ap, msg_ap_fn(0, w), axis=AX.X, op=ALU.add)


def _exp_lrelu(nc, wk, e, shape, tag):
    """ex = exp(leaky_relu(e, NEG)) on the Scalar engine, in place on `e`.
    Prelu and Exp live in the same activation table set (exp_and_others) so
    this costs no table switches.  Fallback (CoreSim lacks Prelu):
    exp(lrelu(x)) = max(exp(x), exp(NEG*x)) since exp is monotonic."""
    if USE_PRELU:
        nc.scalar.activation(e[:], e[:], AF.Prelu, alpha=NEG)
        nc.scalar.activation(e[:], e[:], AF.Exp)
        return e
    ea = wk.tile(shape, BF16, tag=tag + "_a")
    nc.scalar.activation(ea[:], e[:], AF.Exp)
    nc.scalar.activation(e[:], e[:], AF.Exp, scale=NEG)
    nc.vector.tensor_tensor(e[:], e[:], ea[:], op=ALU.max)
    return e


# --------------------------------------------------------------------------
# K1: node tables.  Per column-group g of W node columns the output block is
# [80, W] bf16: rows 0:64 xp1 = x @ W1, 64:72 s1 (att_src dot), 72:80 ad1.
# Input x arrives pre-tiled: per group a contiguous [128, 2, W] fp32 block
# (feature f = c*128 + p), so each group is one contiguous DMA.
# --------------------------------------------------------------------------
def build_k1(groups1):
    tot = sum(2 * P * w for _, w in groups1) * 0 + sum(F_IN * w for _, w in groups1)
    nc = bacc.Bacc("TRN2", target_bir_lowering=False, debug=False, num_devices=NC)
    xg = nc.dram_tensor("xg", [tot], F32, kind="ExternalInput")
    w1 = nc.dram_tensor("w1", [F_IN, HD1], F32, kind="ExternalInput")
    as1 = nc.dram_tensor("as1", [H1, D1], F32, kind="ExternalInput")
    ad1 = nc.dram_tensor("ad1", [H1, D1], F32, kind="ExternalInput")
    out = nc.dram_tensor("q1", [80 * NPC], BF16, kind="ExternalOutput")

    with tile.TileContext(nc) as tc:
        with (
            tc.tile_pool(name="pro", bufs=1) as pro,
            tc.tile_pool(name="io", bufs=3) as io,
            tc.tile_pool(name="ps", bufs=4, space="PSUM") as ps,
        ):
            w1sb = pro.tile([P, 2, HD1], F32)
            nc.sync.dma_start(w1sb[:], w1[:].rearrange("(c p) d -> p c d", p=P))
            asr = _rep_row(nc, pro, as1, P, HD1, "asr")
            adr = _rep_row(nc, pro, ad1, P, HD1, "adr")

            # w_s1[f, h] = sum_d W1[f, h*8+d] * att_src1[h, d]; same for dst
            wext = pro.tile([P, 2, 80], F32)
            nc.scalar.copy(wext[:, :, 0:HD1], w1sb[:])
            for att, lo in ((asr, 64), (adr, 72)):
                tmp = pro.tile([P, 2, HD1], F32, tag="k1tmp")
                nc.vector.tensor_tensor(
                    tmp[:], w1sb[:], _bc(att[:], [P, 2, HD1]), op=ALU.mult
                )
                nc.vector.tensor_reduce(
                    wext[:, :, lo : lo + 8],
                    tmp[:].rearrange("p c (h d) -> p c h d", d=D1),
                    axis=AX.X,
                    op=ALU.add,
                )
            wextb = pro.tile([P, 2, 80], BF16)
            nc.vector.tensor_copy(wextb[:], wext[:])

            oin = 0
            oout = 0
            for _t0, w in groups1:
                xt = io.tile([P, 2, w], F32, tag="xt")
                nc.sync.dma_start(
                    xt[:],
                    xg[oin : oin + F_IN * w].rearrange("(p f) -> p f", f=2 * w),
                )
                oin += F_IN * w
                xb = io.tile([P, 2, w], BF16, tag="xb")
                nc.vector.tensor_copy(xb[:], xt[:])
                pt = ps.tile([80, w], F32, tag="k1ps")
                nc.tensor.matmul(
                    pt[:], lhsT=wextb[:, 0, :], rhs=xb[:, 0, :],
                    start=True, stop=False,
                )
                nc.tensor.matmul(
                    pt[:], lhsT=wextb[:, 1, :], rhs=xb[:, 1, :],
                    start=False, stop=True,
                )
                ot = io.tile([80, w], BF16, tag="k1o")
                nc.scalar.copy(ot[:], pt[:])
                nc.sync.dma_start(
                    out[oout : oout + 80 * w].rearrange("(r n) -> r n", n=w),
                    ot[:],
                )
                oout += 80 * w
    nc.compile()
    return nc


# --------------------------------------------------------------------------
# K2: layer-1 edge aggregation + ELU + fused xp2/s2/ad2 table.
#   Stream per group (all bf16), per partition row:
#     [ xp1 blocks (G*64*K) | s1 blocks (G*8*K) | ad1 blocks (G*8*K) ]
#   p-major slots: slot = base + p*(G*K) + g*K + k.  Pad slots have
#   s1 = -1e38 (exp -> 0) and xp = 0.
#   out per group: [18, G*128] bf16 (rows 0:16 xp2, 16 s2, 17 ad2).
# --------------------------------------------------------------------------
def build_k2(groups):
    tot_in = sum(P * g * 80 * kb for _, g, kb in groups)
    tot_out = sum(18 * g * P for _, g, _ in groups)
    nc = bacc.Bacc("TRN2", target_bir_lowering=False, debug=False, num_devices=NC)
    ev = nc.dram_tensor("ev1", [tot_in], BF16, kind="ExternalInput")
    w2 = nc.dram_tensor("w2", [HD1, D2], F32, kind="ExternalInput")
    as2 = nc.dram_tensor("as2", [1, D2], F32, kind="ExternalInput")
    ad2 = nc.dram_tensor("ad2", [1, D2], F32, kind="ExternalInput")
    b1t = nc.dram_tensor("b1", [HD1], F32, kind="ExternalInput")
    out = nc.dram_tensor("t2", [tot_out], BF16, kind="ExternalOutput")

    from concourse.masks import make_identity

    with tile.TileContext(nc) as tc:
        with (
            tc.tile_pool(name="pro", bufs=1) as pro,
            tc.tile_pool(name="st", bufs=2) as st,
            tc.tile_pool(name="wk", bufs=2) as wk,
            tc.tile_pool(name="mg", bufs=1) as mg,
            tc.tile_pool(name="ps", bufs=2, space="PSUM") as ps,
        ):
            w2sb = pro.tile([HD1, D2], F32)
            nc.sync.dma_start(w2sb[:], w2[:])
            a2s = _rep_row(nc, pro, as2, HD1, D2, "a2s")
            a2d = _rep_row(nc, pro, ad2, HD1, D2, "a2d")
            b1r = _rep_row(nc, pro, b1t, P, HD1, "b1r")
            identf = pro.tile([P, P], F32)
            make_identity(nc, identf[:])
            identb = pro.tile([P, P], BF16)
            nc.vector.tensor_copy(identb[:], identf[:])

            # W2ext [64, 18] = [W2 | W2@att_src2 | W2@att_dst2], bf16
            w2e = pro.tile([HD1, 18], F32)
            nc.scalar.copy(w2e[:, 0:D2], w2sb[:])
            for att, col in ((a2s, 16), (a2d, 17)):
                tmp2 = pro.tile([HD1, D2], F32, tag="k2tmp")
                nc.vector.tensor_tensor(tmp2[:], w2sb[:], att[:], op=ALU.mult)
                nc.vector.tensor_reduce(
                    w2e[:, col : col + 1], tmp2[:], axis=AX.X, op=ALU.add
                )
            w2eb = pro.tile([HD1, 18], BF16)
            nc.vector.tensor_copy(w2eb[:], w2e[:])

            def light(oin, t0, G, K):
                rowlen = G * 80 * K
                evt = st.tile([P, rowlen], BF16, tag="evt")
                nc.sync.dma_start(
                    evt[:],
                    ev[oin : oin + P * rowlen].rearrange("(p f) -> p f", f=rowlen),
                )
                s1v = evt[:, G * 64 * K : G * 72 * K].rearrange(
                    "p (gh k) -> p gh k", k=K
                )
                adv = evt[:, G * 72 * K : G * 80 * K].rearrange(
                    "p (gh k) -> p gh k", k=K
                )
                e = wk.tile([P, G * 8, K], BF16, tag="e")
                nc.vector.tensor_tensor(e[:], s1v, adv, op=ALU.add)
                ex = _exp_lrelu(nc, wk, e, [P, G * 8, K], "ex")
                return evt, ex

            def heavy(evt, ex, t0, G, K):
                xpv = evt[:, 0 : G * 64 * K].rearrange(
                    "p (gh d k) -> p gh d k", d=D1, k=K
                )
                msg = mg.tile([P, G * 8, D1, K], BF16, tag="msg")
                nc.vector.tensor_tensor(
                    msg[:], xpv, _mid0(ex[:], 2, D1), op=ALU.mult
                )
                # denominators from ex (in place), then 1/(dn+eps)
                dn = wk.tile([P, G * 8], F32, tag="dn")
                _tree_reduce(nc, lambda lo, w: ex[:, :, lo : lo + w], dn[:], K)
                nc.vector.tensor_scalar_add(dn[:], dn[:], 1e-16)
                inv = wk.tile([P, G * 8], F32, tag="inv")
                nc.vector.reciprocal_approx_fast(inv[:], dn[:])
                # aggregate messages
                agg = wk.tile([P, G * 8, D1], F32, tag="agg")
                _tree_reduce(nc, lambda lo, w: msg[:, :, :, lo : lo + w], agg[:], K)
                # y = agg * inv + b1 ; h = elu(y) in bf16
                nc.vector.tensor_tensor(
                    agg[:], agg[:], _tail0(inv[:], D1), op=ALU.mult
                )
                yv = agg[:].rearrange("p (g h) d -> p g (h d)", g=G)
                nc.vector.tensor_tensor(yv, yv, _bc(b1r[:], [P, G, HD1]), op=ALU.add)
                hpos = wk.tile([P, G, HD1], F32, tag="hpos")
                nc.scalar.activation(hpos[:], yv, AF.Relu)
                nc.vector.tensor_tensor(yv, yv, hpos[:], op=ALU.subtract)
                nc.scalar.activation(yv, yv, AF.Exp)  # exp(min(y, 0))
                nc.vector.tensor_tensor(yv, yv, hpos[:], op=ALU.add)
                h = wk.tile([P, G, HD1], BF16, tag="h")
                nc.vector.tensor_scalar_add(h[:], yv, -1.0)
                # xp2/s2/ad2 via per-tile transpose + one matmul per 512 cols
                phT = ps.tile([HD1, G * P], BF16, tag="phT")
                for gi in range(G):
                    nc.tensor.transpose(
                        phT[:, gi * P : (gi + 1) * P], h[:, gi, :], identb[:]
                    )
                hTs = wk.tile([HD1, G * P], BF16, tag="hTs")
                nc.scalar.copy(hTs[:], phT[:])
                pt2 = ps.tile([18, G * P], F32, tag="pt2")
                for off in range(0, G * P, 512):
                    wdt = min(512, G * P - off)
                    nc.tensor.matmul(
                        pt2[:, off : off + wdt],
                        lhsT=w2eb[:],
                        rhs=hTs[:, off : off + wdt],
                        start=True,
                        stop=True,
                    )
                st2 = st.tile([18, G * P], BF16, tag="st2")
                nc.scalar.copy(st2[:], pt2[:])
                return st2

            # software-pipelined: group i's attention (DVE add + ACT) is
            # issued before group i-1's heavy DVE block so the engines
            # overlap.
            oin = 0
            oout = 0
            prev = None
            for t0, G, K in groups:
                cur = (*light(oin, t0, G, K), t0, G, K)
                oin += P * G * 80 * K
                if prev is not None:
                    evt, ex, pt0, pG, pK = prev
                    st2 = heavy(evt, ex, pt0, pG, pK)
                    nc.sync.dma_start(
                        out[oout : oout + 18 * pG * P].rearrange(
                            "(r n) -> r n", n=pG * P
                        ),
                        st2[:],
                    )
                    oout += 18 * pG * P
                prev = cur
            evt, ex, pt0, pG, pK = prev
            st2 = heavy(evt, ex, pt0, pG, pK)
            nc.sync.dma_start(
                out[oout : oout + 18 * pG * P].rearrange("(r n) -> r n", n=pG * P),
                st2[:],
            )
    nc.compile()
    return nc


# --------------------------------------------------------------------------
# K3: layer-2 edge aggregation + bias + log_softmax.
#   Stream per group (bf16), per partition row:
#     [ xp2 blocks (G*16*K) | s2 blocks (G*K) | ad2 blocks (G*K) ]
#   out per group: [128, G, 16] fp32 (final log_softmax rows).
# --------------------------------------------------------------------------
def build_k3(groups):
    tot_in = sum(P * g * 18 * kb for _, g, kb in groups)
    tot_out = sum(P * g * D2 for _, g, _ in groups)
    nc = bacc.Bacc("TRN2", target_bir_lowering=False, debug=False, num_devices=NC)
    ev = nc.dram_tensor("ev2", [tot_in], BF16, kind="ExternalInput")
    b2t = nc.dram_tensor("b2", [D2], F32, kind="ExternalInput")
    out = nc.dram_tensor("o3", [tot_out], F32, kind="ExternalOutput")

    with tile.TileContext(nc) as tc:
        with (
            tc.tile_pool(name="pro", bufs=1) as pro,
            tc.tile_pool(name="st", bufs=2) as st,
            tc.tile_pool(name="wk", bufs=2) as wk,
            tc.tile_pool(name="mg", bufs=1) as mg,
        ):
            b2r = _rep_row(nc, pro, b2t, P, D2, "b2r")

            def light(oin, G, K):
                rowlen = G * 18 * K
                evt = st.tile([P, rowlen], BF16, tag="evt")
                nc.sync.dma_start(
                    evt[:],
                    ev[oin : oin + P * rowlen].rearrange("(p f) -> p f", f=rowlen),
                )
                s2v = evt[:, G * 16 * K : G * 17 * K].rearrange(
                    "p (g k) -> p g k", k=K
                )
                adv = evt[:, G * 17 * K : G * 18 * K].rearrange(
                    "p (g k) -> p g k", k=K
                )
                e = wk.tile([P, G, K], BF16, tag="e")
                nc.vector.tensor_tensor(e[:], s2v, adv, op=ALU.add)
                ex = _exp_lrelu(nc, wk, e, [P, G, K], "ex")
                return evt, ex

            def heavy(evt, ex, G, K):
                xpv = evt[:, 0 : G * 16 * K].rearrange(
                    "p (g d k) -> p g d k", d=D2, k=K
                )
                msg = mg.tile([P, G, D2, K], BF16, tag="msg")
                nc.vector.tensor_tensor(
                    msg[:], xpv, _mid0(ex[:], 2, D2), op=ALU.mult
                )
                dn = wk.tile([P, G], F32, tag="dn")
                _tree_reduce(nc, lambda lo, w: ex[:, :, lo : lo + w], dn[:], K)
                nc.vector.tensor_scalar_add(dn[:], dn[:], 1e-16)
                inv = wk.tile([P, G], F32, tag="inv")
                nc.vector.reciprocal_approx_fast(inv[:], dn[:])
                o = wk.tile([P, G, D2], F32, tag="o")
                _tree_reduce(nc, lambda lo, w: msg[:, :, :, lo : lo + w], o[:], K)
                nc.vector.tensor_tensor(o[:], o[:], _tail0(inv[:], D2), op=ALU.mult)
                nc.vector.tensor_tensor(
                    o[:], o[:], _bc(b2r[:], [P, G, D2]), op=ALU.add
                )
                # log_softmax over the 16 classes
                nm = wk.tile([P, G], F32, tag="nm")
                nc.vector.tensor_reduce(
                    nm[:], o[:], axis=AX.X, op=ALU.max, negate=True
                )
                nc.vector.tensor_tensor(o[:], o[:], _tail0(nm[:], D2), op=ALU.add)
                exq = wk.tile([P, G, D2], F32, tag="exq")
                nc.scalar.activation(exq[:], o[:], AF.Exp)
                ss = wk.tile([P, G], F32, tag="ss")
                nc.vector.tensor_reduce(ss[:], exq[:], axis=AX.X, op=ALU.add)
                nc.scalar.activation(ss[:], ss[:], AF.Ln)
                ov = wk.tile([P, G, D2], F32, tag="ov")
                nc.vector.tensor_tensor(
                    ov[:], o[:], _tail0(ss[:], D2), op=ALU.subtract
                )
                return ov

            oin = 0
            oout = 0
            prev = None
            for t0, G, K in groups:
                cur = (*light(oin, G, K), G, K)
                oin += P * G * 18 * K
                if prev is not None:
                    evt, ex, pG, pK = prev
                    ov = heavy(evt, ex, pG, pK)
                    nc.sync.dma_start(
                        out[oout : oout + P * pG * D2].rearrange(
                            "(p f) -> p f", f=pG * D2
                        ),
                        ov[:],
                    )
                    oout += P * pG * D2
                prev = cur
            evt, ex, pG, pK = prev
            ov = heavy(evt, ex, pG, pK)
            nc.sync.dma_start(
                out[oout : oout + P * pG * D2].rearrange("(p f) -> p f", f=pG * D2),
                ov[:],
            )
    nc.compile()
    return nc


# --------------------------------------------------------------------------
# Host orchestration
# --------------------------------------------------------------------------
def _even4(k):
    """Round a tile's K up so the halving tree stays 4B-aligned."""
    k = int(k)
    if k <= 2:
        return 2
    if k <= 4:
        return 4
    return 4 * ((k + 3) // 4)


def _make_groups(k_step, gmax, slot_budget):
    """Greedy: grow the group while tiles*K stays under slot_budget."""
    groups = []
    t0 = 0
    while t0 < STEPS:
        g = 1
        kb = _even4(k_step[t0])
        while t0 + g < STEPS and g < gmax:
            nkb = max(kb, _even4(k_step[t0 + g]))
            if (g + 1) * nkb > slot_budget:
                break
            kb = nkb
            g += 1
        groups.append((t0, g, kb))
        t0 += g
    return groups


def _build_slots(groups, spos_node, deg, estart, src_by_dst):
    """slot -> src node id (N = pad) per core; layout per group is p-major:
    slot = base + p*(G*K) + g*K + k."""
    tot = sum(P * g * kb for _, g, kb in groups)
    slot = np.full((NC, tot), N, dtype=np.int64)
    arangeP = np.arange(P)
    for c in range(NC):
        base = 0
        for t0, g, kb in groups:
            for gi in range(g):
                T = (t0 + gi) * NC + c
                nodes = spos_node[T * P : (T + 1) * P]
                valid = nodes >= 0
                nv = nodes[valid]
                if nv.size == 0:
                    continue
                d = deg[nv]
                rowstart = base + arangeP[valid] * (g * kb) + gi * kb
                totd = int(d.sum())
                if totd == 0:
                    continue
                rep_row = np.repeat(rowstart, d)
                rep_cum = np.repeat(np.cumsum(d) - d, d)
                intra = np.arange(totd) - rep_cum
                rep_est = np.repeat(estart[nv], d)
                slot[c, rep_row + intra] = src_by_dst[rep_est + intra]
            base += P * g * kb
    return slot


def kernel(x, edge_index, W1, att_src1, att_dst1, b1, W2, att_src2, att_dst2, b2):
    x = np.asarray(x, dtype=np.float32)
    edge_index = np.asarray(edge_index)
    W1 = np.asarray(W1, dtype=np.float32)
    att_src1 = np.asarray(att_src1, dtype=np.float32)
    att_dst1 = np.asarray(att_dst1, dtype=np.float32)
    b1 = np.asarray(b1, dtype=np.float32)
    W2 = np.asarray(W2, dtype=np.float32)
    att_src2 = np.asarray(att_src2, dtype=np.float32).reshape(1, D2)
    att_dst2 = np.asarray(att_dst2, dtype=np.float32).reshape(1, D2)
    b2 = np.asarray(b2, dtype=np.float32)

    src = edge_index[0].astype(np.int64)
    dst = edge_index[1].astype(np.int64)

    # ---- schedule: degree-sorted tiles, round-robin dealt across cores ----
    deg = np.bincount(dst, minlength=N)
    order = np.argsort(deg, kind="stable")          # sorted-node space -> node id
    eo = np.argsort(dst, kind="stable")             # edges sorted by dst
    src_by_dst = src[eo]
    estart = np.zeros(N + 1, dtype=np.int64)
    estart[1:] = np.cumsum(deg)

    spos_node = np.full(TILES * P, -1, dtype=np.int64)
    spos_node[:N] = order
    sdeg = np.zeros(TILES * P, dtype=np.int64)
    sdeg[:N] = deg[order]
    tile_max = sdeg.reshape(TILES, P).max(axis=1)
    k_step = np.maximum(tile_max.reshape(STEPS, NC).max(axis=1), 1)  # [STEPS]

    groups2 = _make_groups(k_step, 8, 288)
    groups3 = _make_groups(k_step, 16, 768)
    slots2 = _build_slots(groups2, spos_node, deg, estart, src_by_dst)
    slots3 = _build_slots(groups3, spos_node, deg, estart, src_by_dst)
    ad_rows = np.where(spos_node < 0, N, spos_node)  # [TILES*P] node per row
    # per-core view: row t*128+p of core c <-> sorted pos (t*NC+c)*128+p
    ad_rows = (
        ad_rows.reshape(STEPS, NC, P).transpose(1, 0, 2).reshape(NC, NPC)
    )

    # ---- K1: node tables ----
    groups1 = []
    t0 = 0
    while t0 < STEPS:
        g = min(4, STEPS - t0)
        groups1.append((t0, g * P))
        t0 += g
    xpad = np.zeros((NC * NPC, F_IN), dtype=np.float32)
    xpad[:N] = x
    nc1 = build_k1(groups1)
    in1 = []
    for c in range(NC):
        xt = np.ascontiguousarray(xpad[c * NPC : (c + 1) * NPC].T)  # [256, NPC]
        blocks = []
        for t0, w in groups1:
            col = t0 * P
            blk = xt[:, col : col + w].reshape(2, P, w).transpose(1, 0, 2)
            blocks.append(blk.ravel())
        in1.append(
            {
                "xg": np.concatenate(blocks),
                "w1": W1,
                "as1": att_src1,
                "ad1": att_dst1,
            }
        )
    r1 = _run(nc1, in1, "k1")

    # xq1 row-major per-core table (+1 shared pad row), uint16-viewed bf16
    xq1 = np.zeros((NC * NPC + 1, 80), dtype=np.uint16)
    for c in range(NC):
        q = _u16(r1[c]["q1"])
        o = 0
        for t0, w in groups1:
            xq1[c * NPC + t0 * P : c * NPC + t0 * P + w] = (
                q[o : o + 80 * w].reshape(80, w).T
            )
            o += 80 * w
    PAD_BF = _u16(np.asarray([PADS], BF_U16))[0]
    xq1[-1, 64:72] = PAD_BF                         # pad row: s1 = -1e38, rest 0

    # ---- K2: layer 1 ----
    nc2 = build_k2(groups2)
    pad2 = np.where(slots2 >= N, NC * NPC, slots2)
    ad_pad = np.where(ad_rows >= N, NC * NPC, ad_rows)

    def _stream2(c):
        parts = []
        base = 0
        for t0, g, kb in groups2:
            n = P * g * kb
            rows = xq1[pad2[c, base : base + n]].reshape(P, g, kb, 80)
            base += n
            xp = (
                rows[..., 0:64]
                .reshape(P, g, kb, 8, 8)
                .transpose(0, 1, 3, 4, 2)
                .reshape(P, g * 64 * kb)
            )
            s1 = rows[..., 64:72].transpose(0, 1, 3, 2).reshape(P, g * 8 * kb)
            adn = ad_pad[c, t0 * P : (t0 + g) * P].reshape(g, P)
            adv = xq1[adn, 72:80].transpose(1, 0, 2)          # [P, g, 8]
            adv = np.broadcast_to(adv[..., None], (P, g, 8, kb)).reshape(
                P, g * 8 * kb
            )
            parts.append(
                np.concatenate([xp, s1, adv], axis=1).ravel()
            )
        return _bf(np.concatenate(parts))

    in2 = [
        {
            "ev1": _stream2(c),
            "w2": W2,
            "as2": att_src2,
            "ad2": att_dst2,
            "b1": b1,
        }
        for c in range(NC)
    ]
    r2 = _run(nc2, in2, "k2")

    # reassemble layer-2 node table in original-node space (uint16 bf16)
    t2 = np.zeros((N + 1, 18), dtype=np.uint16)
    t2[N, 16] = PAD_BF                              # pad row: s2 = -1e38
    for c in range(NC):
        q = _u16(r2[c]["t2"])
        o = 0
        for t0, g, kb in groups2:
            blk = q[o : o + 18 * g * P].reshape(18, g, P)
            o += 18 * g * P
            for gi in range(g):
                T = (t0 + gi) * NC + c
                nodes = spos_node[T * P : (T + 1) * P]
                valid = nodes >= 0
                t2[nodes[valid]] = blk[:, gi, :].T[valid]

    # ---- K3: layer 2 ----
    nc3 = build_k3(groups3)
    pad3 = np.where(slots3 >= N, N, slots3)
    ad_pad3 = np.where(ad_rows >= N, N, ad_rows)

    def _stream3(c):
        parts = []
        base = 0
        for t0, g, kb in groups3:
            n = P * g * kb
            rows = t2[pad3[c, base : base + n]].reshape(P, g, kb, 18)
            base += n
            xp = (
                rows[..., 0:16].transpose(0, 1, 3, 2).reshape(P, g * 16 * kb)
            )
            s2 = rows[..., 16].reshape(P, g * kb)
            adn = ad_pad3[c, t0 * P : (t0 + g) * P].reshape(g, P)
            adv = t2[adn, 17].transpose(1, 0)                 # [P, g]
            adv = np.broadcast_to(adv[..., None], (P, g, kb)).reshape(P, g * kb)
            parts.append(np.concatenate([xp, s2, adv], axis=1).ravel())
        return _bf(np.concatenate(parts))

    in3 = [{"ev2": _stream3(c), "b2": b2} for c in range(NC)]
    r3 = _run(nc3, in3, "k3")

    outp = np.zeros((N, D2), dtype=np.float32)
    for c in range(NC):
        q = np.asarray(r3[c]["o3"], dtype=np.float32)
        o = 0
        for t0, g, kb in groups3:
            blk = q[o : o + P * g * D2].reshape(P, g, D2)
            o += P * g * D2
            for gi in range(g):
                T = (t0 + gi) * NC + c
                nodes = spos_node[T * P : (T + 1) * P]
                valid = nodes >= 0
                outp[nodes[valid]] = blk[:, gi, :][valid]
    return outp


# revision 19
# speedup vs baseline: 1.5804x; 1.1186x over previous
"""GAT 2-layer network on 8 Trainium2 NeuronCores.

Strategy (edge-parallel, per the sharding hint "partition edges, replicate
node features"):
  - Nodes are sorted by in-degree and packed into 128-node tiles; tiles are
    dealt round-robin onto the 8 cores so every core runs the identical
    instruction stream (SPMD) over a shared per-step K schedule.
  - All FLOPs run on device across 3 launches:
      K1: xp1 = x @ W1 plus per-head attention dot products (s1, ad1),
          emitted in bf16.
      K2: per dst-tile segment softmax + message aggregation for layer 1,
          ELU, then xp2 = h @ W2ext (fused) -> layer-2 node table (bf16).
      K3: layer-2 segment softmax + aggregation + bias + log_softmax (fp32
          out).
  - Between launches the host only does index-based data movement: it
    replicates the device-computed per-node tables into per-edge-slot
    streams (degree-padded, p-major, bf16 moved as uint16) so each device
    step reads one contiguous DMA per group.  No floating-point math
    happens on the host.
  - Perf notes vs the first version of this kernel: all edge streams are
    bf16 (halves HBM traffic AND unlocks the DVE 2x_1p mode for
    tensor_tensor), segment reductions use in-place halving trees of
    2x-mode TT adds instead of the never-accelerated tensor_reduce, the
    leaky-relu+exp runs on the Scalar engine (Prelu+Exp share one
    activation table set), and each group's inputs arrive as a single
    contiguous DMA.
"""

import os
import sys

for _p in ("/opt/trn_rl_repo", "/root/.axon_site/_ro/trn_rl_repo"):
    if os.path.isdir(_p) and _p not in sys.path:
        sys.path.insert(0, _p)

import ml_dtypes
import numpy as np

import concourse.bacc as bacc
import concourse.bass as bass
import concourse.tile as tile
from concourse import mybir
from concourse.bass_utils import run_bass_kernel_spmd

F32 = mybir.dt.float32
BF16 = mybir.dt.bfloat16
AF = mybir.ActivationFunctionType
ALU = mybir.AluOpType
AX = mybir.AxisListType

N = 100000
E = 1600000
F_IN = 256
H1, D1 = 8, 8
HD1 = H1 * D1          # 64
D2 = 16                # H2 = 1
NEG = 0.2
NC = 8
P = 128
TILES = 784            # ceil(100000 / 128) rounded up to a multiple of 8
STEPS = TILES // NC    # 98
NPC = STEPS * P        # 12544 node rows handled per core in K1
PADS = -1.0e38         # sentinel: exp(lrelu(PADS + ad)) == 0 exactly

USE_PRELU = True       # Prelu not implemented by CoreSim; HW supports it
STOP_AFTER = int(os.environ.get("STOP_AFTER", "3"))   # debug: 1/2 = early out
TRACE = False          # test.py flips this for NTFF profiling
SIM = False            # run through CoreSim instead of hardware
SIM_CORES = None       # e.g. [0] to only simulate core 0
LAST_EXEC_NS = []      # per-launch exec_time_ns when TRACE

BF_U16 = np.dtype(ml_dtypes.bfloat16)


def _bf(x):
    """View a uint16 array as bfloat16 (no conversion)."""
    return np.asarray(x).view(BF_U16)


def _u16(x):
    """View a bfloat16-ish 2-byte array as uint16 (no conversion)."""
    x = np.asarray(x)
    assert x.dtype.itemsize == 2, x.dtype
    return x.view(np.uint16)


def _run(nc, in_maps, tag):
    if SIM:
        from concourse.bass_interp import CoreSim

        outs = []
        cores = range(NC) if SIM_CORES is None else SIM_CORES
        for c in range(NC):
            if c not in cores:
                outs.append(outs[-1] if outs else {})
                continue
            sim = CoreSim(nc, trace=False)
            for k, v in in_maps[c].items():
                sim.tensor(k)[:] = v
            sim.simulate(check_with_hw=False)
            onames = [
                a.memorylocations[0].name
                for a in nc.m.functions[0].allocations
                if isinstance(a, mybir.MemoryLocationSet) and a.kind == "ExternalOutput"
            ]
            outs.append({k: np.array(sim.tensor(k)) for k in onames})
        return outs
    if TRACE:
        import hookfix  # noqa: F401  (registers antenv.axon_hooks)

        hookfix.install()
    res = run_bass_kernel_spmd(nc, in_maps, list(range(NC)), trace=TRACE)
    if TRACE:
        LAST_EXEC_NS.append((tag, res.exec_time_ns))
    return res.results


def _bc(ap, shape):
    """Broadcast the free dims of `ap` to `shape` (partition dim must already
    match).  Target dims are matched against source free dims right-to-left;
    size-1 source dims and unmatched target dims become step-0 (broadcast)."""
    src = ap.ap
    assert src[0][1] == shape[0], (src, shape)
    sdims = list(src[1:])
    res = []
    si = len(sdims) - 1
    for ti in range(len(shape) - 1, 0, -1):
        if si >= 0 and sdims[si][1] == shape[ti]:
            res.append(sdims[si])
            si -= 1
        elif si >= 0 and sdims[si][1] == 1:
            res.append([0, shape[ti]])
            si -= 1
        else:
            res.append([0, shape[ti]])
    assert si < 0, (src, shape)
    return bass.AP(tensor=ap.tensor, offset=ap.offset, ap=[src[0]] + res[::-1])


def _tail0(ap, n):
    """Append a trailing step-0 (broadcast) dim of size n."""
    return bass.AP(tensor=ap.tensor, offset=ap.offset, ap=list(ap.ap) + [[0, n]])


def _mid0(ap, pos, n):
    """Insert a step-0 (broadcast) dim of size n at free-dim position pos
    (ap.ap index pos, counting the partition dim as 0)."""
    dims = list(ap.ap)
    return bass.AP(
        tensor=ap.tensor, offset=ap.offset, ap=dims[:pos] + [[0, n]] + dims[pos:]
    )


def _rep_row(nc, pool, dram_t, nparts, cols, tag):
    """DMA-replicate a flat `cols`-element DRAM tensor across `nparts`
    partitions (engines cannot broadcast across partitions themselves)."""
    tl = pool.tile([nparts, cols], F32, tag=tag)
    src = bass.AP(tensor=dram_t[:].tensor, offset=0, ap=[[0, nparts], [1, cols]])
    nc.sync.dma_start(tl[:], src)
    return tl


def _tree_reduce(nc, rng, idx, out_ap, kb):
    """Reduce the innermost (k) axis of a degree-padded block with in-place
    TT adds that all hit the DVE 2x_1p mode: first fold the tail down onto
    the largest power of two <= kb (in1 offset = p2 elements, even, so every
    2-byte row stays 4B aligned), then halve, then emit the final width-2 add
    straight into `out_ap` (fp32).  `rng(lo, w)` is the AP covering k in
    [lo, lo+w); `idx(k)` the AP with the k axis dropped."""
    w = kb
    p2 = 1 << (w.bit_length() - 1)
    if w > p2:
        r = w - p2
        nc.vector.tensor_tensor(rng(0, r), rng(0, r), rng(p2, r), op=ALU.add)
        w = p2
    while w > 2:
        h2 = w // 2
        nc.vector.tensor_tensor(rng(0, h2), rng(0, h2), rng(h2, h2), op=ALU.add)
        w = h2
    if w == 2:
        nc.vector.tensor_tensor(out_ap, idx(0), idx(1), op=ALU.add)
    else:
        nc.vector.tensor_copy(out_ap, idx(0))


def _exp_lrelu(nc, wk, e, shape, tag):
    """ex = exp(leaky_relu(e, NEG)) on the Scalar engine, in place on `e`.
    Prelu and Exp live in the same activation table set (exp_and_others) so
    this costs no table switches.  Fallback (CoreSim lacks Prelu):
    exp(lrelu(x)) = max(exp(x), exp(NEG*x)) since exp is monotonic."""
    if USE_PRELU:
        nc.scalar.activation(e[:], e[:], AF.Prelu, alpha=NEG)
        nc.scalar.activation(e[:], e[:], AF.Exp)
        return e
    ea = wk.tile(shape, BF16, tag=tag + "_a")
    nc.scalar.activation(ea[:], e[:], AF.Exp)
    nc.scalar.activation(e[:], e[:], AF.Exp, scale=NEG)
    nc.vector.tensor_tensor(e[:], e[:], ea[:], op=ALU.max)
    return e


# --------------------------------------------------------------------------
# K1: node tables.  Per column-group g of W node columns the output block is
# [80, W] bf16: rows 0:64 xp1 = x @ W1, 64:72 s1 (att_src dot), 72:80 ad1.
# Input x arrives pre-tiled: per group a contiguous [128, 2, W] fp32 block
# (feature f = c*128 + p), so each group is one contiguous DMA.
# --------------------------------------------------------------------------
def build_k1(groups1):
    tot = sum(2 * P * w for _, w in groups1) * 0 + sum(F_IN * w for _, w in groups1)
    nc = bacc.Bacc("TRN2", target_bir_lowering=False, debug=False, num_devices=NC)
    xg = nc.dram_tensor("xg", [tot], F32, kind="ExternalInput")
    w1 = nc.dram_tensor("w1", [F_IN, HD1], F32, kind="ExternalInput")
    as1 = nc.dram_tensor("as1", [H1, D1], F32, kind="ExternalInput")
    ad1 = nc.dram_tensor("ad1", [H1, D1], F32, kind="ExternalInput")
    out = nc.dram_tensor("q1", [80 * NPC], BF16, kind="ExternalOutput")

    with tile.TileContext(nc) as tc:
        with (
            tc.tile_pool(name="pro", bufs=1) as pro,
            tc.tile_pool(name="io", bufs=3) as io,
            tc.tile_pool(name="ps", bufs=2, space="PSUM") as ps,
        ):
            w1sb = pro.tile([P, 2, HD1], F32)
            nc.sync.dma_start(w1sb[:], w1[:].rearrange("(c p) d -> p c d", p=P))
            asr = _rep_row(nc, pro, as1, P, HD1, "asr")
            adr = _rep_row(nc, pro, ad1, P, HD1, "adr")

            # w_s1[f, h] = sum_d W1[f, h*8+d] * att_src1[h, d]; same for dst
            wext = pro.tile([P, 2, 80], F32)
            nc.scalar.copy(wext[:, :, 0:HD1], w1sb[:])
            for att, lo in ((asr, 64), (adr, 72)):
                tmp = pro.tile([P, 2, HD1], F32, tag="k1tmp")
                nc.vector.tensor_tensor(
                    tmp[:], w1sb[:], _bc(att[:], [P, 2, HD1]), op=ALU.mult
                )
                nc.vector.tensor_reduce(
                    wext[:, :, lo : lo + 8],
                    tmp[:].rearrange("p c (h d) -> p c h d", d=D1),
                    axis=AX.X,
                    op=ALU.add,
                )
            wextb = pro.tile([P, 2, 80], BF16)
            nc.vector.tensor_copy(wextb[:], wext[:])

            oin = 0
            oout = 0
            for _t0, w in groups1:
                xt = io.tile([P, 2, w], F32, tag="xt")
                nc.sync.dma_start(
                    xt[:],
                    xg[oin : oin + F_IN * w].rearrange("(p f) -> p f", f=2 * w),
                )
                oin += F_IN * w
                xb = io.tile([P, 2, w], BF16, tag="xb")
                nc.vector.tensor_copy(xb[:], xt[:])
                pt = ps.tile([80, w], F32, tag="k1ps")
                for c in (0, 1):
                    for off in range(0, w, 512):
                        wdt = min(512, w - off)
                        nc.tensor.matmul(
                            pt[:, off : off + wdt],
                            lhsT=wextb[:, c, :],
                            rhs=xb[:, c, off : off + wdt],
                            start=(c == 0),
                            stop=(c == 1),
                        )
                ot = io.tile([80, w], BF16, tag="k1o")
                nc.scalar.copy(ot[:], pt[:])
                nc.scalar.dma_start(
                    out[oout : oout + 80 * w].rearrange("(r n) -> r n", n=w),
                    ot[:],
                )
                oout += 80 * w
    nc.compile()
    return nc


# --------------------------------------------------------------------------
# K2: layer-1 edge aggregation + ELU + fused xp2/s2/ad2 table.
#   Stream per group (all bf16), per partition row:
#     [ xp1 blocks (G*64*K) | s1 blocks (G*8*K) | ad1 blocks (G*8*K) ]
#   p-major slots: slot = base + p*(G*K) + g*K + k.  Pad slots have
#   s1 = -1e38 (exp -> 0) and xp = 0.
#   out per group: [18, G*128] bf16 (rows 0:16 xp2, 16 s2, 17 ad2).
# --------------------------------------------------------------------------
def build_k2(groups):
    tot_in = sum(P * g * 80 * kb for _, g, kb in groups)
    tot_out = sum(18 * g * P for _, g, _ in groups)
    nc = bacc.Bacc("TRN2", target_bir_lowering=False, debug=False, num_devices=NC)
    ev = nc.dram_tensor("ev1", [tot_in], BF16, kind="ExternalInput")
    w2 = nc.dram_tensor("w2", [HD1, D2], F32, kind="ExternalInput")
    as2 = nc.dram_tensor("as2", [1, D2], F32, kind="ExternalInput")
    ad2 = nc.dram_tensor("ad2", [1, D2], F32, kind="ExternalInput")
    b1t = nc.dram_tensor("b1", [HD1], F32, kind="ExternalInput")
    out = nc.dram_tensor("t2", [tot_out], BF16, kind="ExternalOutput")

    from concourse.masks import make_identity

    with tile.TileContext(nc) as tc:
        with (
            tc.tile_pool(name="pro", bufs=1) as pro,
            tc.tile_pool(name="st", bufs=2) as st,
            tc.tile_pool(name="wk", bufs=2) as wk,
            tc.tile_pool(name="mg", bufs=1) as mg,
            tc.tile_pool(name="ps", bufs=2, space="PSUM") as ps,
        ):
            w2sb = pro.tile([HD1, D2], F32)
            nc.sync.dma_start(w2sb[:], w2[:])
            a2s = _rep_row(nc, pro, as2, HD1, D2, "a2s")
            a2d = _rep_row(nc, pro, ad2, HD1, D2, "a2d")
            b1r = _rep_row(nc, pro, b1t, P, HD1, "b1r")
            identf = pro.tile([P, P], F32)
            make_identity(nc, identf[:])
            identb = pro.tile([P, P], BF16)
            nc.vector.tensor_copy(identb[:], identf[:])

            # W2ext [64, 18] = [W2 | W2@att_src2 | W2@att_dst2], bf16
            w2e = pro.tile([HD1, 18], F32)
            nc.scalar.copy(w2e[:, 0:D2], w2sb[:])
            for att, col in ((a2s, 16), (a2d, 17)):
                tmp2 = pro.tile([HD1, D2], F32, tag="k2tmp")
                nc.vector.tensor_tensor(tmp2[:], w2sb[:], att[:], op=ALU.mult)
                nc.vector.tensor_reduce(
                    w2e[:, col : col + 1], tmp2[:], axis=AX.X, op=ALU.add
                )
            w2eb = pro.tile([HD1, 18], BF16)
            nc.vector.tensor_copy(w2eb[:], w2e[:])
            c_m1 = pro.tile([P, 1], F32)
            nc.vector.memset(c_m1[:], -1.0)

            def light(oin, t0, G, K):
                rowlen = G * 80 * K
                evt = st.tile([P, rowlen], BF16, tag="evt")
                nc.sync.dma_start(
                    evt[:],
                    ev[oin : oin + P * rowlen].rearrange("(p f) -> p f", f=rowlen),
                )
                s1v = evt[:, G * 64 * K : G * 72 * K].rearrange(
                    "p (gh k) -> p gh k", k=K
                )
                adv = evt[:, G * 72 * K : G * 80 * K].rearrange(
                    "p (gh k) -> p gh k", k=K
                )
                e = wk.tile([P, G * 8, K], BF16, tag="e")
                nc.vector.tensor_tensor(e[:], s1v, adv, op=ALU.add)
                ex = _exp_lrelu(nc, wk, e, [P, G * 8, K], "ex")
                return evt, ex

            def heavy(evt, ex, t0, G, K):
                xpv = evt[:, 0 : G * 64 * K].rearrange(
                    "p (gh d k) -> p gh d k", d=D1, k=K
                )
                msg = mg.tile([P, G * 8, D1, K], BF16, tag="msg")
                nc.vector.tensor_tensor(
                    msg[:], xpv, _mid0(ex[:], 2, D1), op=ALU.mult
                )
                # denominators from ex (in place), then 1/(dn+eps)
                dn = wk.tile([P, G * 8], F32, tag="dn")
                _tree_reduce(
                    nc, lambda lo, w: ex[:, :, lo : lo + w],
                    lambda k: ex[:, :, k], dn[:], K,
                )
                nc.vector.tensor_scalar_add(dn[:], dn[:], 1e-16)
                inv = wk.tile([P, G * 8], F32, tag="inv")
                nc.vector.reciprocal_approx_fast(inv[:], dn[:])
                # aggregate messages
                agg = wk.tile([P, G * 8, D1], F32, tag="agg")
                _tree_reduce(
                    nc, lambda lo, w: msg[:, :, :, lo : lo + w],
                    lambda k: msg[:, :, :, k], agg[:], K,
                )
                # y = agg * inv + b1 ; h = elu(y) = exp(min(y,0)) + (y -
                # min(y,0)) - 1, in bf16.  Structured so the only DVE wait is
                # a short one on the single Scalar-engine exp; the final -1
                # and the bf16 cast ride an ACT Identity+bias.
                nc.vector.tensor_tensor(
                    agg[:], agg[:], _tail0(inv[:], D1), op=ALU.mult
                )
                yv = agg[:].rearrange("p (g h) d -> p g (h d)", g=G)
                nc.vector.tensor_tensor(yv, yv, _bc(b1r[:], [P, G, HD1]), op=ALU.add)
                hneg = wk.tile([P, G, HD1], F32, tag="hneg")
                nc.vector.tensor_scalar_min(hneg[:], yv, 0.0)
                eh = wk.tile([P, G, HD1], F32, tag="eh")
                nc.scalar.activation(eh[:], hneg[:], AF.Exp)
                nc.vector.tensor_tensor(yv, yv, hneg[:], op=ALU.subtract)
                nc.vector.tensor_tensor(yv, yv, eh[:], op=ALU.add)
                h = wk.tile([P, G, HD1], BF16, tag="h")
                nc.scalar.activation(h[:], yv, AF.Identity, bias=c_m1[:])
                # xp2/s2/ad2 via per-tile transpose + one matmul per 512 cols
                phT = ps.tile([HD1, G * P], BF16, tag="phT")
                for gi in range(G):
                    nc.tensor.transpose(
                        phT[:, gi * P : (gi + 1) * P], h[:, gi, :], identb[:]
                    )
                hTs = wk.tile([HD1, G * P], BF16, tag="hTs")
                nc.scalar.copy(hTs[:], phT[:])
                pt2 = ps.tile([18, G * P], F32, tag="pt2")
                for off in range(0, G * P, 512):
                    wdt = min(512, G * P - off)
                    nc.tensor.matmul(
                        pt2[:, off : off + wdt],
                        lhsT=w2eb[:],
                        rhs=hTs[:, off : off + wdt],
                        start=True,
                        stop=True,
                    )
                st2 = st.tile([18, G * P], BF16, tag="st2")
                nc.scalar.copy(st2[:], pt2[:])
                return st2

            # software-pipelined: group i's attention (DVE add + ACT) is
            # issued before group i-1's heavy DVE block so the engines
            # overlap.
            oin = 0
            oout = 0
            prev = None
            for t0, G, K in groups:
                cur = (*light(oin, t0, G, K), t0, G, K)
                oin += P * G * 80 * K
                if prev is not None:
                    evt, ex, pt0, pG, pK = prev
                    st2 = heavy(evt, ex, pt0, pG, pK)
                    nc.scalar.dma_start(
                        out[oout : oout + 18 * pG * P].rearrange(
                            "(r n) -> r n", n=pG * P
                        ),
                        st2[:],
                    )
                    oout += 18 * pG * P
                prev = cur
            evt, ex, pt0, pG, pK = prev
            st2 = heavy(evt, ex, pt0, pG, pK)
            nc.scalar.dma_start(
                out[oout : oout + 18 * pG * P].rearrange("(r n) -> r n", n=pG * P),
                st2[:],
            )
    nc.compile()
    return nc


# --------------------------------------------------------------------------
# K3: layer-2 edge aggregation + bias + log_softmax.
#   Stream per group (bf16), per partition row:
#     [ xp2 blocks (G*16*K) | s2 blocks (G*K) | ad2 blocks (G*K) ]
#   out per group: [128, G, 16] fp32 (final log_softmax rows).
# --------------------------------------------------------------------------
def build_k3(groups):
    tot_in = sum(P * g * 18 * kb for _, g, kb in groups)
    nc = bacc.Bacc("TRN2", target_bir_lowering=False, debug=False, num_devices=NC)
    ev = nc.dram_tensor("ev2", [tot_in], BF16, kind="ExternalInput")
    b2t = nc.dram_tensor("b2", [D2], F32, kind="ExternalInput")
    out = nc.dram_tensor("o3", [NPC * D2], F32, kind="ExternalOutput")

    with tile.TileContext(nc) as tc:
        with (
            tc.tile_pool(name="pro", bufs=1) as pro,
            tc.tile_pool(name="st", bufs=2) as st,
            tc.tile_pool(name="wk", bufs=2) as wk,
            tc.tile_pool(name="mg", bufs=1) as mg,
        ):
            b2r = _rep_row(nc, pro, b2t, P, D2, "b2r")
            # persistent accumulators: shifted logits for every step tile and
            # the per-node softmax sums.  The Ln + final subtraction + the
            # single output DMA happen once in the epilogue, so no Ln table
            # reload per group (exp/prelu/ln don't share a set with each
            # other's defaults otherwise).
            oall = pro.tile([P, STEPS, D2], F32, tag="oall")
            ssal = pro.tile([P, STEPS], F32, tag="ssal")

            def light(oin, G, K):
                rowlen = G * 18 * K
                evt = st.tile([P, rowlen], BF16, tag="evt")
                nc.sync.dma_start(
                    evt[:],
                    ev[oin : oin + P * rowlen].rearrange("(p f) -> p f", f=rowlen),
                )
                s2v = evt[:, G * 16 * K : G * 17 * K].rearrange(
                    "p (g k) -> p g k", k=K
                )
                adv = evt[:, G * 17 * K : G * 18 * K].rearrange(
                    "p (g k) -> p g k", k=K
                )
                e = wk.tile([P, G, K], BF16, tag="e")
                nc.vector.tensor_tensor(e[:], s2v, adv, op=ALU.add)
                ex = _exp_lrelu(nc, wk, e, [P, G, K], "ex")
                return evt, ex

            def heavy(evt, ex, t0, G, K):
                xpv = evt[:, 0 : G * 16 * K].rearrange(
                    "p (g d k) -> p g d k", d=D2, k=K
                )
                msg = mg.tile([P, G, D2, K], BF16, tag="msg")
                nc.vector.tensor_tensor(
                    msg[:], xpv, _mid0(ex[:], 2, D2), op=ALU.mult
                )
                dn = wk.tile([P, G], F32, tag="dn")
                _tree_reduce(
                    nc, lambda lo, w: ex[:, :, lo : lo + w],
                    lambda k: ex[:, :, k], dn[:], K,
                )
                nc.vector.tensor_scalar_add(dn[:], dn[:], 1e-16)
                inv = wk.tile([P, G], F32, tag="inv")
                nc.vector.reciprocal_approx_fast(inv[:], dn[:])
                o = oall[:, t0 : t0 + G, :]
                _tree_reduce(
                    nc, lambda lo, w: msg[:, :, :, lo : lo + w],
                    lambda k: msg[:, :, :, k], o, K,
                )
                # normalization + bias, then the softmax denominator.  No
                # max-shift needed: the logits are bounded (|o| < ~10) so
                # fp32 exp cannot overflow, and the final log_softmax only
                # differs by fp rounding.
                nc.vector.tensor_tensor(o, o, _tail0(inv[:], D2), op=ALU.mult)
                nc.vector.tensor_tensor(o, o, _bc(b2r[:], [P, G, D2]), op=ALU.add)
                exq = wk.tile([P, G, D2], F32, tag="exq")
                nc.scalar.activation(exq[:], o, AF.Exp)
                nc.vector.tensor_reduce(
                    ssal[:, t0 : t0 + G], exq[:], axis=AX.X, op=ALU.add
                )

            oin = 0
            prev = None
            for t0, G, K in groups:
                cur = (*light(oin, G, K), t0, G, K)
                oin += P * G * 18 * K
                if prev is not None:
                    evt, ex, pt0, pG, pK = prev
                    heavy(evt, ex, pt0, pG, pK)
                prev = cur
            evt, ex, pt0, pG, pK = prev
            heavy(evt, ex, pt0, pG, pK)

            # epilogue: one Ln over all step tiles, one subtraction, one DMA
            nc.scalar.activation(ssal[:], ssal[:], AF.Ln)
            nc.vector.tensor_tensor(
                oall[:], oall[:], _tail0(ssal[:], D2), op=ALU.subtract
            )
            nc.sync.dma_start(
                out[:].rearrange("(p f) -> p f", f=STEPS * D2), oall[:]
            )
    nc.compile()
    return nc


# --------------------------------------------------------------------------
# Host orchestration
# --------------------------------------------------------------------------
def _even2(k):
    """Round a tile's K up to even so bf16 rows stay 4B-aligned."""
    k = int(k)
    return max(2, k + (k & 1))


def _make_groups(k_step, gmax, slot_budget):
    """Greedy: grow the group while tiles*K stays under slot_budget."""
    groups = []
    t0 = 0
    while t0 < STEPS:
        g = 1
        kb = _even2(k_step[t0])
        while t0 + g < STEPS and g < gmax:
            nkb = max(kb, _even2(k_step[t0 + g]))
            if (g + 1) * nkb > slot_budget:
                break
            kb = nkb
            g += 1
        groups.append((t0, g, kb))
        t0 += g
    return groups


def _build_slots(groups, spos_node, deg, estart, src_by_dst):
    """slot -> src node id (N = pad) per core; layout per group is p-major:
    slot = base + p*(G*K) + g*K + k."""
    tot = sum(P * g * kb for _, g, kb in groups)
    slot = np.full((NC, tot), N, dtype=np.int64)
    arangeP = np.arange(P)
    for c in range(NC):
        base = 0
        for t0, g, kb in groups:
            for gi in range(g):
                T = (t0 + gi) * NC + c
                nodes = spos_node[T * P : (T + 1) * P]
                valid = nodes >= 0
                nv = nodes[valid]
                if nv.size == 0:
                    continue
                d = deg[nv]
                rowstart = base + arangeP[valid] * (g * kb) + gi * kb
                totd = int(d.sum())
                if totd == 0:
                    continue
                rep_row = np.repeat(rowstart, d)
                rep_cum = np.repeat(np.cumsum(d) - d, d)
                intra = np.arange(totd) - rep_cum
                rep_est = np.repeat(estart[nv], d)
                slot[c, rep_row + intra] = src_by_dst[rep_est + intra]
            base += P * g * kb
    return slot


def kernel(x, edge_index, W1, att_src1, att_dst1, b1, W2, att_src2, att_dst2, b2):
    x = np.asarray(x, dtype=np.float32)
    edge_index = np.asarray(edge_index)
    W1 = np.asarray(W1, dtype=np.float32)
    att_src1 = np.asarray(att_src1, dtype=np.float32)
    att_dst1 = np.asarray(att_dst1, dtype=np.float32)
    b1 = np.asarray(b1, dtype=np.float32)
    W2 = np.asarray(W2, dtype=np.float32)
    att_src2 = np.asarray(att_src2, dtype=np.float32).reshape(1, D2)
    att_dst2 = np.asarray(att_dst2, dtype=np.float32).reshape(1, D2)
    b2 = np.asarray(b2, dtype=np.float32)

    src = edge_index[0].astype(np.int64)
    dst = edge_index[1].astype(np.int64)

    # ---- schedule: degree-sorted tiles, round-robin dealt across cores ----
    deg = np.bincount(dst, minlength=N)
    order = np.argsort(deg, kind="stable")          # sorted-node space -> node id
    eo = np.argsort(dst, kind="stable")             # edges sorted by dst
    src_by_dst = src[eo]
    estart = np.zeros(N + 1, dtype=np.int64)
    estart[1:] = np.cumsum(deg)

    spos_node = np.full(TILES * P, -1, dtype=np.int64)
    spos_node[:N] = order
    sdeg = np.zeros(TILES * P, dtype=np.int64)
    sdeg[:N] = deg[order]
    tile_max = sdeg.reshape(TILES, P).max(axis=1)
    k_step = np.maximum(tile_max.reshape(STEPS, NC).max(axis=1), 1)  # [STEPS]

    groups2 = _make_groups(k_step, 8, 288)
    groups3 = _make_groups(k_step, 16, 768)
    slots2 = _build_slots(groups2, spos_node, deg, estart, src_by_dst)
    slots3 = _build_slots(groups3, spos_node, deg, estart, src_by_dst)
    ad_rows = np.where(spos_node < 0, N, spos_node)  # [TILES*P] node per row
    # per-core view: row t*128+p of core c <-> sorted pos (t*NC+c)*128+p
    ad_rows = (
        ad_rows.reshape(STEPS, NC, P).transpose(1, 0, 2).reshape(NC, NPC)
    )

    # ---- K1: node tables ----
    groups1 = []
    t0 = 0
    while t0 < STEPS:
        g = min(16, STEPS - t0)
        groups1.append((t0, g * P))
        t0 += g
    xpad = np.zeros((NC * NPC, F_IN), dtype=np.float32)
    xpad[:N] = x
    nc1 = build_k1(groups1)
    in1 = []
    for c in range(NC):
        xt = np.ascontiguousarray(xpad[c * NPC : (c + 1) * NPC].T)  # [256, NPC]
        blocks = []
        for t0, w in groups1:
            col = t0 * P
            blk = xt[:, col : col + w].reshape(2, P, w).transpose(1, 0, 2)
            blocks.append(blk.ravel())
        in1.append(
            {
                "xg": np.concatenate(blocks),
                "w1": W1,
                "as1": att_src1,
                "ad1": att_dst1,
            }
        )
    r1 = _run(nc1, in1, "k1")
    if STOP_AFTER == 1:
        return np.zeros((N, D2), dtype=np.float32)

    # xq1 row-major per-core table (+1 shared pad row), uint16-viewed bf16
    xq1 = np.zeros((NC * NPC + 1, 80), dtype=np.uint16)
    for c in range(NC):
        q = _u16(r1[c]["q1"])
        o = 0
        for t0, w in groups1:
            xq1[c * NPC + t0 * P : c * NPC + t0 * P + w] = (
                q[o : o + 80 * w].reshape(80, w).T
            )
            o += 80 * w
    PAD_BF = _u16(np.asarray([PADS], BF_U16))[0]
    xq1[-1, 64:72] = PAD_BF                         # pad row: s1 = -1e38, rest 0

    # ---- K2: layer 1 ----
    nc2 = build_k2(groups2)
    pad2 = np.where(slots2 >= N, NC * NPC, slots2)
    ad_pad = np.where(ad_rows >= N, NC * NPC, ad_rows)

    def _stream2(c):
        parts = []
        base = 0
        for t0, g, kb in groups2:
            n = P * g * kb
            rows = xq1[pad2[c, base : base + n]].reshape(P, g, kb, 80)
            base += n
            xp = (
                rows[..., 0:64]
                .reshape(P, g, kb, 8, 8)
                .transpose(0, 1, 3, 4, 2)
                .reshape(P, g * 64 * kb)
            )
            s1 = rows[..., 64:72].transpose(0, 1, 3, 2).reshape(P, g * 8 * kb)
            adn = ad_pad[c, t0 * P : (t0 + g) * P].reshape(g, P)
            adv = xq1[adn, 72:80].transpose(1, 0, 2)          # [P, g, 8]
            adv = np.broadcast_to(adv[..., None], (P, g, 8, kb)).reshape(
                P, g * 8 * kb
            )
            parts.append(
                np.concatenate([xp, s1, adv], axis=1).ravel()
            )
        return _bf(np.concatenate(parts))

    in2 = [
        {
            "ev1": _stream2(c),
            "w2": W2,
            "as2": att_src2,
            "ad2": att_dst2,
            "b1": b1,
        }
        for c in range(NC)
    ]
    r2 = _run(nc2, in2, "k2")
    if STOP_AFTER == 2:
        return np.zeros((N, D2), dtype=np.float32)

    # reassemble layer-2 node table in original-node space (uint16 bf16)
    t2 = np.zeros((N + 1, 18), dtype=np.uint16)
    t2[N, 16] = PAD_BF                              # pad row: s2 = -1e38
    for c in range(NC):
        q = _u16(r2[c]["t2"])
        o = 0
        for t0, g, kb in groups2:
            blk = q[o : o + 18 * g * P].reshape(18, g, P)
            o += 18 * g * P
            for gi in range(g):
                T = (t0 + gi) * NC + c
                nodes = spos_node[T * P : (T + 1) * P]
                valid = nodes >= 0
                t2[nodes[valid]] = blk[:, gi, :].T[valid]

    # ---- K3: layer 2 ----
    nc3 = build_k3(groups3)
    pad3 = np.where(slots3 >= N, N, slots3)
    ad_pad3 = np.where(ad_rows >= N, N, ad_rows)

    def _stream3(c):
        parts = []
        base = 0
        for t0, g, kb in groups3:
            n = P * g * kb
            rows = t2[pad3[c, base : base + n]].reshape(P, g, kb, 18)
            base += n
            xp = (
                rows[..., 0:16].transpose(0, 1, 3, 2).reshape(P, g * 16 * kb)
            )
            s2 = rows[..., 16].reshape(P, g * kb)
            adn = ad_pad3[c, t0 * P : (t0 + g) * P].reshape(g, P)
            adv = t2[adn, 17].transpose(1, 0)                 # [P, g]
            adv = np.broadcast_to(adv[..., None], (P, g, kb)).reshape(P, g * kb)
            parts.append(np.concatenate([xp, s2, adv], axis=1).ravel())
        return _bf(np.concatenate(parts))

    in3 = [{"ev2": _stream3(c), "b2": b2} for c in range(NC)]
    r3 = _run(nc3, in3, "k3")

    outp = np.zeros((N, D2), dtype=np.float32)
    for c in range(NC):
        q = np.asarray(r3[c]["o3"], dtype=np.float32).reshape(P, STEPS, D2)
        for t in range(STEPS):
            T = t * NC + c
            nodes = spos_node[T * P : (T + 1) * P]
            valid = nodes >= 0
            outp[nodes[valid]] = q[:, t, :][valid]
    return outp
